# revision 35
# baseline (speedup 1.0000x reference)
"""BysMamba Trainium2 kernel v2: 8-core SPMD bass/Tile implementation.

Sharding: core c = (batch b = c//4) x (d_inner shard s = c%4, 128 channels).
Replica groups [[0..3],[4..7]]. The fp32 residual h (256 x 2048 per batch,
replicated within each group) lives in SBUF for the whole kernel.

v2 structure (vs v1):
- in_proj+conv replicated: every core computes xi for ALL 512 channels (its
  conv weights are reordered so ch-tile 0 is the core's own shard), which
  makes x_proj fully local and removes the per-pass 48x2048 AllReduce.
- Each directional pass is split into two L-halves (1024 tokens) that are
  software-pipelined: while DVE runs the 16-state scan loop of half i, the
  PE/Act head of half i+2 runs, and the out-proj AllReduce + h-update of
  half i-1 rides DMA (cce-add with bf16->f32 cast) under the scans.
- Engine roles: DVE = scans (the only engine that can run them) + most
  dbx/gn mults + y2; Pool(gpsimd) = a few mults + cast/accum DMAs; Act =
  silu/exp/ln/dA; PE = all matmuls incl. the state-sum via identity lhsT
  and d_param*xi via diagonal lhsT.
- Per-layer weights streamed from DRAM, double-buffered, prefetched one
  layer ahead.
"""
import sys
import os

for _p in ("/opt/trn_rl_repo", "/root/.axon_site/_ro/trn_rl_repo"):
    if os.path.isdir(_p) and _p not in sys.path:
        sys.path.insert(0, _p)

import numpy as np
import ml_dtypes

import concourse.bass as bass
import concourse.tile as tile
from concourse import mybir
from concourse.bass_utils import run_bass_kernel_spmd

BF = ml_dtypes.bfloat16
F32 = mybir.dt.float32
BF16 = mybir.dt.bfloat16
I32 = mybir.dt.int32

B = 2
L = 2048
HC = 1024          # half-chunk length
DIM = 256
DIN = 512
NST = 16
VOCAB = 474
NM = 10
LPAD = 3
LT = L + LPAD
LSH = 512
NCORES = 8
GROUPS = [[0, 1, 2, 3], [4, 5, 6, 7]]
NPOOL = 5          # states whose dbx/gn mults run on Pool

_prog_cache = {}


def _split_excess_waits(nc, max_waits=1):
    """walrus here rejects >1 sync-wait per instruction; split the excess
    onto same-engine NoOps placed immediately before."""
    n = 0
    for fn in nc.m.functions:
        for blk in fn.blocks:
            out = []
            changed = False
            for inst in blk.instructions:
                si = inst.sync_info
                waits = list(si.on_wait) if si is not None and si.on_wait else []
                if len(waits) > max_waits:
                    extra = waits[:-max_waits]
                    si.on_wait = waits[-max_waits:]
                    for i in range(0, len(extra), max_waits):
                        out.append(mybir.InstNoOp(
                            name=f"{inst.name}-wsplit-{i}",
                            engine=inst.engine, ins=[], outs=[],
                            sync_info=mybir.SyncInfo(
                                on_wait=extra[i:i + max_waits], on_update=[]),
                        ))
                        n += 1
                    changed = True
                out.append(inst)
            if changed:
                blk.instructions = out
    return n


def _bcast_row_ap(dram_tile_ap, row, col0, width):
    """AP reading one DRAM row segment replicated across 128 partitions."""
    r = dram_tile_ap[row:row + 1, col0:col0 + width]
    return bass.AP(tensor=r.tensor, offset=r.offset, ap=[[0, 128], [1, width]])


def _build_program(a_scales):
    AOP = mybir.AluOpType
    AF = mybir.ActivationFunctionType

    nc = bass.Bass(num_devices=NCORES)

    def par(name, shape, dt):
        return nc.declare_dram_parameter(name, list(shape), dt, isOutput=False)

    t9 = par("t9", (9 * VOCAB, DIM), F32)
    idxp = par("idxp", (128, 36), I32)
    wconvD = par("wconvD", (128, NM * 1024), BF16)
    wzD = par("wzD", (128, NM * 256), BF16)
    wxD = par("wxD", (128, NM * 48), BF16)
    wdtD = par("wdtD", (16, NM * 128), BF16)
    woutD = par("woutD", (128, NM * 256), BF16)
    wdprmD = par("wdprmD", (128, NM * 128), BF16)
    lmh = par("lmh", (128, 2 * VOCAB), BF16)
    bdtp = par("bdt", (128, NM), F32)
    cbp = par("cb", (128, NM), F32)
    b9p = par("b9", (128, 2), F32)
    identb = par("identb", (128, 128), BF16)
    identf = par("identf", (128, 128), F32)

    logits = nc.declare_dram_parameter("logits", [VOCAB, L], F32, isOutput=True)

    import contextlib
    with tile.TileContext(nc) as tc, contextlib.ExitStack() as ctx:
        persist = ctx.enter_context(tc.tile_pool(name="persist", bufs=1))
        wst = ctx.enter_context(tc.tile_pool(name="wst", bufs=2))
        ps = ctx.enter_context(tc.tile_pool(name="ps", bufs=2, space="PSUM"))
        bc = ctx.enter_context(tc.tile_pool(name="bc", bufs=4))
        wk = ctx.enter_context(tc.tile_pool(name="wk", bufs=2))
        fe = ctx.enter_context(tc.tile_pool(name="fe", bufs=3))
        dram = ctx.enter_context(tc.tile_pool(name="dram", bufs=3, space="DRAM"))

        def ld(param, shape, dt, tag):
            t = persist.tile(list(shape), dt, tag=tag, name=tag)
            nc.sync.dma_start(out=t[:], in_=param[:])
            return t

        lmh_s = ld(lmh, (128, 2 * VOCAB), BF16, "lmh_s")
        bdt_s = ld(bdtp, (128, NM), F32, "bdt_s")
        cb_s = ld(cbp, (128, NM), F32, "cb_s")
        b9_s = ld(b9p, (128, 2), F32, "b9_s")
        idb_s = ld(identb, (128, 128), BF16, "idb_s")
        idf_s = ld(identf, (128, 128), F32, "idf_s")
        idx_s = ld(idxp, (128, 36), I32, "idx_s")

        h32 = [persist.tile([128, LT], F32, tag=f"h32_{k}", name=f"h32_{k}") for k in range(2)]
        hbf = [persist.tile([128, LT], BF16, tag=f"hbf_{k}", name=f"hbf_{k}") for k in range(2)]
        hrv = [persist.tile([128, LT], BF16, tag=f"hrv_{k}", name=f"hrv_{k}") for k in range(2)]
        for k in range(2):
            nc.vector.memset(h32[k][:], 0.0)
            nc.vector.memset(hbf[k][:], 0.0)
            nc.vector.memset(hrv[k][:], 0.0)

        xi_t = persist.tile([128, L], BF16, tag="xi_t", name="xi_t")
        sz_t = persist.tile([128, L], BF16, tag="sz_t", name="sz_t")
        dl_t = persist.tile([128, L], BF16, tag="dl_t", name="dl_t")
        u_t = persist.tile([128, L], BF16, tag="u_t", name="u_t")
        dbc_sb = persist.tile([48, L], BF16, tag="dbc_sb", name="dbc_sb")
        dt_sb = persist.tile([16, L], BF16, tag="dt_sb", name="dt_sb")
        carries = persist.tile([128, NST], F32, tag="carries", name="carries")
        outp = [persist.tile([128, L], BF16, tag=f"outp_{k}", name=f"outp_{k}") for k in range(2)]
        h0loc = [persist.tile([128, LSH], F32, tag=f"h0loc_{k}", name=f"h0loc_{k}") for k in range(2)]

        # ---- per-layer streamed weights ------------------------------------
        wcache = {}

        def load_weights(l):
            if l in wcache or l >= NM:
                return
            w = {}
            w["cv"] = wst.tile([128, 1024], BF16, tag="w_cv", name="w_cv")
            nc.sync.dma_start(out=w["cv"][:], in_=wconvD[:, l * 1024:(l + 1) * 1024])
            w["z"] = wst.tile([128, 256], BF16, tag="w_z", name="w_z")
            nc.sync.dma_start(out=w["z"][:], in_=wzD[:, l * 256:(l + 1) * 256])
            w["x"] = wst.tile([128, 48], BF16, tag="w_x", name="w_x")
            nc.sync.dma_start(out=w["x"][:], in_=wxD[:, l * 48:(l + 1) * 48])
            w["dt"] = wst.tile([16, 128], BF16, tag="w_dt", name="w_dt")
            nc.sync.dma_start(out=w["dt"][:], in_=wdtD[:, l * 128:(l + 1) * 128])
            w["o"] = wst.tile([128, 256], BF16, tag="w_o", name="w_o")
            nc.sync.dma_start(out=w["o"][:], in_=woutD[:, l * 256:(l + 1) * 256])
            w["dp"] = wst.tile([128, 128], BF16, tag="w_dp", name="w_dp")
            nc.sync.dma_start(out=w["dp"][:], in_=wdprmD[:, l * 128:(l + 1) * 128])
            wcache[l] = w

        load_weights(0)

        # ---- front-end -----------------------------------------------------
        ptt = ps.tile([128, HC], F32, tag="ps", name="fe_ps")
        for tau in range(4):
            acc = fe.tile([128, DIM], F32, tag="feacc", name="feacc")
            for j in range(9):
                g = fe.tile([128, DIM], F32, tag="feg", name="feg")
                nc.gpsimd.indirect_dma_start(
                    out=g[:], out_offset=None, in_=t9[:],
                    in_offset=bass.IndirectOffsetOnAxis(
                        ap=idx_s[:, tau * 9 + j: tau * 9 + j + 1], axis=0),
                )
                if j == 0:
                    nc.vector.tensor_copy(out=acc[:], in_=g[:])
                else:
                    nc.vector.tensor_tensor(out=acc[:], in0=acc[:], in1=g[:],
                                             op=AOP.add)
            for dh in range(2):
                blk = tau * 2 + dh
                nc.tensor.transpose(
                    out=ptt[:, blk * 128:(blk + 1) * 128],
                    in_=acc[:, dh * 128:(dh + 1) * 128],
                    identity=idf_s[:])
                nc.vector.tensor_scalar(
                    out=h0loc[dh][:, tau * 128:(tau + 1) * 128],
                    in0=ptt[:, blk * 128:(blk + 1) * 128],
                    scalar1=b9_s[:, dh:dh + 1], scalar2=None, op0=AOP.add)

        agi = dram.tile([2, 128, LSH], F32, tag="agi", name="agi", bufs=1)
        ago = dram.tile([4, 2, 128, LSH], F32, tag="ago", name="ago", bufs=1)
        for k in range(2):
            nc.sync.dma_start(out=agi[k], in_=h0loc[k][:])
        nc.gpsimd.collective_compute(
            "AllGather", AOP.bypass, replica_groups=GROUPS,
            ins=[agi.opt()], outs=[ago.opt()])
        for g in range(4):
            for k in range(2):
                nc.sync.dma_start(
                    out=h32[k][:, LPAD + g * LSH: LPAD + (g + 1) * LSH],
                    in_=ago[g, k])
        for k in range(2):
            nc.vector.tensor_copy(out=hbf[k][:], in_=h32[k][:])

        # ---- half-pass building blocks ------------------------------------

        def head(l, hb, c, first_of_layer):
            """PE/Act head for half c: in_proj(own 128 ch) -> silu -> z ->
            x_proj partial into dbc_sb. No DVE ops."""
            w = wcache[l]
            if first_of_layer:
                load_weights(l + 1)
            c0 = c * HC
            pxc = ps.tile([128, HC], F32, tag="ps", name="pxc")
            for kt in range(2):
                for j in range(4):
                    lt = w["cv"][:, (j * 2 + kt) * 128:
                                 (j * 2 + kt) * 128 + 128]
                    for nt in range(2):
                        nc.tensor.matmul(
                            out=pxc[:, nt * 512:(nt + 1) * 512],
                            lhsT=lt,
                            rhs=hb[kt][:, c0 + nt * 512 + j:
                                       c0 + nt * 512 + j + 512],
                            start=(kt == 0 and j == 0),
                            stop=(kt == 1 and j == 3))
            nc.scalar.activation(out=xi_t[:, c0:c0 + HC], in_=pxc[:],
                                 func=AF.Silu,
                                 bias=cb_s[:, l:l + 1], scale=1.0)
            pz = ps.tile([128, HC], F32, tag="ps", name="pz")
            for nt in range(2):
                for kt in range(2):
                    nc.tensor.matmul(
                        out=pz[:, nt * 512:(nt + 1) * 512],
                        lhsT=w["z"][:, kt * 128:(kt + 1) * 128],
                        rhs=hb[kt][:, LPAD + c0 + nt * 512:
                                   LPAD + c0 + nt * 512 + 512],
                        start=(kt == 0), stop=(kt == 1))
            nc.scalar.activation(out=sz_t[:, c0:c0 + HC], in_=pz[:],
                                 func=AF.Silu, scale=1.0)
            pxp = ps.tile([128, HC], F32, tag="ps", name="pxp")
            for nt in range(2):
                nc.tensor.matmul(
                    out=pxp[:48, nt * 512:(nt + 1) * 512],
                    lhsT=w["x"][:],
                    rhs=xi_t[:, c0 + nt * 512: c0 + nt * 512 + 512],
                    start=True, stop=True)
            nc.scalar.copy(out=dbc_sb[:, c0:c0 + HC], in_=pxp[:48, :])

        def finish_head(l):
            """After both halves' x_proj partials: AllReduce dbc over the
            d_inner shards, then dt -> softplus for both halves."""
            w = wcache[l]
            dbcI = dram.tile([48, L], BF16, tag="dbcI", name="dbcI")
            nc.sync.dma_start(out=dbcI[:], in_=dbc_sb[:])
            dbcD = dram.tile([48, L], BF16, tag="dbcD", name="dbcD")
            nc.gpsimd.collective_compute(
                "AllReduce", AOP.add, replica_groups=GROUPS,
                ins=[dbcI.opt()], outs=[dbcD.opt()])
            nc.sync.dma_start(out=dt_sb[:], in_=dbcD[:16, :])
            for c in range(2):
                c0 = c * HC
                pdt = ps.tile([128, HC], F32, tag="ps", name="pdt")
                for nt in range(2):
                    nc.tensor.matmul(
                        out=pdt[:, nt * 512:(nt + 1) * 512],
                        lhsT=w["dt"][:],
                        rhs=dt_sb[:, c0 + nt * 512: c0 + nt * 512 + 512],
                        start=True, stop=True)
                e_b = wk.tile([128, HC], BF16, tag="e_b", name="e_b")
                nc.scalar.activation(out=e_b[:], in_=pdt[:], func=AF.Exp,
                                     bias=bdt_s[:, l:l + 1], scale=1.0)
                nc.scalar.activation(out=dl_t[:, c0:c0 + HC], in_=e_b[:],
                                     func=AF.Ln, bias=1.0, scale=1.0)
            return dbcD

        def da_block(l, c):
            """16 Act exps for half c (deep-buffered da tiles)."""
            c0 = c * HC
            das = []
            for n in range(NST):
                da = wk.tile([128, HC], BF16, tag="da", name="da", bufs=10)
                nc.scalar.activation(out=da[:], in_=dl_t[:, c0:c0 + HC],
                                     func=AF.Exp, scale=float(a_scales[l][n]))
                das.append(da)
            return das

        def states_tail(l, c, dbcD, das, oinD):
            """DVE/Pool state loop for half c + y2/out-proj."""
            w = wcache[l]
            c0 = c * HC
            nc.vector.tensor_tensor(out=u_t[:, c0:c0 + HC],
                                    in0=dl_t[:, c0:c0 + HC],
                                    in1=xi_t[:, c0:c0 + HC], op=AOP.mult)
            py = ps.tile([128, HC], F32, tag="py", name="py")
            for n in range(NST):
                pool_state = (n >= NST - NPOOL)
                eng = nc.gpsimd if pool_state else nc.vector
                sfx = "p" if pool_state else "v"
                bbc = bc.tile([128, HC], BF16, tag="bbc", name="bbc")
                nc.sync.dma_start(out=bbc[:],
                                  in_=_bcast_row_ap(dbcD, 16 + n, c0, HC))
                cbc = bc.tile([128, HC], BF16, tag="cbc", name="cbc")
                nc.sync.dma_start(out=cbc[:],
                                  in_=_bcast_row_ap(dbcD, 32 + n, c0, HC))
                dbx = wk.tile([128, HC], BF16, tag=f"dbx_{sfx}", name="dbx")
                eng.tensor_tensor(out=dbx[:], in0=u_t[:, c0:c0 + HC],
                                  in1=bbc[:], op=AOP.mult)
                hn = wk.tile([128, HC], BF16, tag="hn", name="hn")
                nc.vector.tensor_tensor_scan(
                    out=hn[:], data0=das[n][:], data1=dbx[:],
                    initial=(0.0 if c == 0 else carries[:, n:n + 1]),
                    op0=AOP.mult, op1=AOP.add)
                if c == 0:
                    nc.vector.tensor_copy(out=carries[:, n:n + 1],
                                          in_=hn[:, HC - 1:HC])
                gn = wk.tile([128, HC], BF16, tag=f"gn_{sfx}", name="gn")
                eng.tensor_tensor(out=gn[:], in0=hn[:], in1=cbc[:],
                                  op=AOP.mult)
                for nt in range(2):
                    nc.tensor.matmul(
                        out=py[:, nt * 512:(nt + 1) * 512],
                        lhsT=idb_s[:],
                        rhs=gn[:, nt * 512:(nt + 1) * 512],
                        start=(n == 0), stop=False)
            for nt in range(2):
                nc.tensor.matmul(
                    out=py[:, nt * 512:(nt + 1) * 512],
                    lhsT=w["dp"][:],
                    rhs=xi_t[:, c0 + nt * 512: c0 + nt * 512 + 512],
                    start=False, stop=(nt == 1))
            y2 = wk.tile([128, HC], BF16, tag="y2", name="y2")
            nc.vector.tensor_tensor(out=y2[:], in0=py[:],
                                    in1=sz_t[:, c0:c0 + HC], op=AOP.mult)
            for mt in range(2):
                po = ps.tile([128, HC], F32, tag="ps", name="po")
                for nt in range(2):
                    nc.tensor.matmul(out=po[:, nt * 512:(nt + 1) * 512],
                                     lhsT=w["o"][:, mt * 128:(mt + 1) * 128],
                                     rhs=y2[:, nt * 512:(nt + 1) * 512],
                                     start=True, stop=True)
                pob = wk.tile([128, HC], BF16, tag="pob", name="pob")
                nc.scalar.copy(out=pob[:], in_=po[:])
                nc.sync.dma_start(out=oinD[mt], in_=pob[:])

        def arh(l, c, oinD, mode):
            """AllReduce of the out halves + h update."""
            c0 = c * HC
            ooutD = dram.tile([2, 128, HC], BF16, tag="ooutD", name="ooutD")
            nc.gpsimd.collective_compute(
                "AllReduce", AOP.add, replica_groups=GROUPS,
                ins=[oinD.opt()], outs=[ooutD.opt()])
            if mode == "fwd":
                for k in range(2):
                    nc.gpsimd.dma_start(
                        out=h32[k][:, LPAD + c0: LPAD + c0 + HC],
                        in_=ooutD[k], accum_op=AOP.add)
                    nc.gpsimd.dma_start(
                        out=hbf[k][:, LPAD + c0: LPAD + c0 + HC],
                        in_=h32[k][:, LPAD + c0: LPAD + c0 + HC])
            else:  # bwd: stage; reversed add at layer end
                for k in range(2):
                    nc.sync.dma_start(out=outp[k][:, c0:c0 + HC],
                                      in_=ooutD[k])

        def bwd_finish():
            for k in range(2):
                nc.vector.tensor_tensor(
                    out=h32[k][:, LPAD:], in0=h32[k][:, LPAD:],
                    in1=outp[k][:, L - 1::-1], op=AOP.add)
                nc.gpsimd.dma_start(out=hbf[k][:, LPAD:],
                                    in_=h32[k][:, LPAD:])

        def refresh_hrv():
            for k in range(2):
                nc.vector.tensor_copy(out=hrv[k][:, LPAD:],
                                      in_=hbf[k][:, LT - 1: LPAD - 1: -1])

        # ---- schedule: 24 half-passes, software-pipelined ------------------
        sched = []
        for l in range(NM):
            bidir = (l == 0 or l == NM - 1)
            if bidir:
                sched += [(l, "fwd", 0), (l, "fwd", 1),
                          (l, "bwd", 0), (l, "bwd", 1)]
            else:
                sched += [(l, "fwd", 0), (l, "fwd", 1)]
        nhp = len(sched)
        last_of_layer = {}
        for i, (l, mode, c) in enumerate(sched):
            last_of_layer[l] = i
        bidir_layers = {0, NM - 1}

        state = {}

        def issue_head(i):
            l, mode, c = sched[i]
            first = (c == 0 and mode == "fwd")
            head(l, hbf if mode == "fwd" else hrv, c, first)
            oinD = dram.tile([2, 128, HC], BF16, tag="oinD", name="oinD")
            if c == 1:
                # both halves' x_proj partials done: reduce + dt chain + dA
                dbcD = finish_head(l)
                das0 = da_block(l, 0)
                das1 = da_block(l, 1)
                _, _, o0 = state[i - 1]
                state[i - 1] = (dbcD, das0, o0)
                state[i] = (dbcD, das1, oinD)
            else:
                state[i] = (None, None, oinD)

        def head_allowed(j, i):
            """head(j) may be issued at end of iteration i?"""
            lj = sched[j][0]
            prev = lj - 1
            if prev >= 0 and prev in bidir_layers and i < last_of_layer[prev]:
                return False
            return True

        next_head = 0

        def pump_heads(i):
            nonlocal next_head
            while next_head < nhp and next_head <= i + 2 \
                    and head_allowed(next_head, i):
                issue_head(next_head)
                next_head += 1

        # l=0 is bidirectional: snapshot the FE h for its bwd passes before
        # any fwd h-update is issued.
        refresh_hrv()
        pump_heads(-1)  # issues heads 0 and 1
        for i in range(nhp):
            l, mode, c = sched[i]
            if l in bidir_layers and mode == "fwd" and c == 0 and l > 0:
                # snapshot the layer input for the bwd passes: the previous
                # layer's h is fully updated (its last arh was issued in the
                # previous iteration) and none of this layer's fwd updates
                # are issued yet.
                refresh_hrv()
            dbcD, das, oinD = state.pop(i)
            states_tail(l, c, dbcD, das, oinD)
            arh(l, c, oinD, mode)
            if mode == "bwd" and c == 1:
                bwd_finish()
            pump_heads(i)

        # ---- lm_head over full L (host slices per core) --------------------
        for mt in range(4):
            m0 = mt * 128
            msz = min(128, VOCAB - m0)
            for nt in range(4):
                plh = ps.tile([128, HC], F32, tag="ps", name="plh")
                for kt in range(2):
                    nc.tensor.matmul(
                        out=plh[:msz, :512],
                        lhsT=lmh_s[:, kt * VOCAB + m0: kt * VOCAB + m0 + msz],
                        rhs=hbf[kt][:, LPAD + nt * 512: LPAD + nt * 512 + 512],
                        start=(kt == 0), stop=(kt == 1))
                lout = wk.tile([128, 512], F32, tag="lout", name="lout")
                nc.vector.tensor_copy(out=lout[:msz, :], in_=plh[:msz, :512])
                nc.sync.dma_start(
                    out=logits[m0:m0 + msz, nt * 512:(nt + 1) * 512],
                    in_=lout[:msz, :])

    return nc


# --------------------------------------------------------------------------
def _host_prep(inputs):
    f = np.float32
    x = np.asarray(inputs["x"]).astype(np.int64).reshape(B, L, 9)
    emb = np.asarray(inputs["emb"], f)
    c2w = np.asarray(inputs["conv2d_w"], f)
    c2b = np.asarray(inputs["conv2d_b"], f)
    w_in = np.asarray(inputs["w_in"], f)
    conv_w = np.asarray(inputs["conv_w"], f)
    conv_b = np.asarray(inputs["conv_b"], f)
    w_x = np.asarray(inputs["w_x"], f)
    w_dt = np.asarray(inputs["w_dt"], f)
    b_dt = np.asarray(inputs["b_dt"], f)
    a_log = np.asarray(inputs["a_log"], f)
    d_param = np.asarray(inputs["d_param"], f)
    w_out = np.asarray(inputs["w_out"], f)
    lm_head = np.asarray(inputs["lm_head"], f)

    t9 = np.empty((9, VOCAB, DIM), f)
    for j in range(9):
        i, jj = divmod(j, 3)
        t9[j] = 0.5 * (emb @ c2w[:, :, i, jj].T)
    t9[4] += 0.5 * emb
    t9f = np.ascontiguousarray(t9.reshape(9 * VOCAB, DIM))
    b9 = 0.5 * c2b

    a_scales = [[float(-np.exp(a_log[l, 0, n])) for n in range(NST)]
                for l in range(NM)]

    per_core = []
    for cid in range(NCORES):
        b, s = divmod(cid, 4)
        ds = slice(128 * s, 128 * s + 128)
        dglob = np.arange(128 * s, 128 * s + 128)

        tok = np.arange(LSH * s, LSH * (s + 1))
        idx = (np.arange(9)[None, :] * VOCAB + x[b][tok]).astype(np.int32)
        idxp = np.zeros((128, 36), np.int32)
        for tau in range(4):
            idxp[:, tau * 9:(tau + 1) * 9] = idx[tau * 128:(tau + 1) * 128]

        wconv = np.zeros((128, NM * 1024), BF)
        wzv = np.zeros((128, NM * 256), BF)
        wxv = np.zeros((128, NM * 48), BF)
        wdtv = np.zeros((16, NM * 128), BF)
        woutv = np.zeros((128, NM * 256), BF)
        wdprmv = np.zeros((128, NM * 128), BF)
        cbv = np.zeros((128, NM), f)
        for l in range(NM):
            wi = w_in[l][:DIN][ds]            # own xi rows (128, 256)
            wzr = w_in[l][DIN:][ds]           # own z rows (128, 256)
            cw = conv_w[l][ds]                # (128, 4)
            for j in range(4):
                for kt in range(2):
                    blkc = l * 1024 + (j * 2 + kt) * 128
                    wconv[:, blkc:blkc + 128] = (
                        cw[:, j][None, :]
                        * wi[:, kt * 128:kt * 128 + 128].T)
            wxv[:, l * 48:(l + 1) * 48] = w_x[l][:, dglob].T
            cbv[:, l] = conv_b[l][dglob]
            for kt in range(2):
                wzv[:, l * 256 + kt * 128: l * 256 + (kt + 1) * 128] = \
                    wzr[:, kt * 128:kt * 128 + 128].T
            wdtv[:, l * 128:(l + 1) * 128] = w_dt[l][dglob].T
            sc = 0.5 if (l == 0 or l == NM - 1) else 1.0
            woutv[:, l * 256:(l + 1) * 256] = sc * w_out[l][:, dglob].T
            wdprmv[:, l * 128:(l + 1) * 128] = np.diag(d_param[l][dglob])

        lmhv = np.zeros((128, 2 * VOCAB), BF)
        for kt in range(2):
            lmhv[:, kt * VOCAB:(kt + 1) * VOCAB] = \
                lm_head[:, kt * 128:(kt + 1) * 128].T

        per_core.append({
            "t9": t9f,
            "idxp": idxp,
            "wconvD": wconv, "wzD": wzv, "wxD": wxv, "wdtD": wdtv,
            "woutD": woutv, "wdprmD": wdprmv,
            "lmh": lmhv,
            "bdt": np.ascontiguousarray(b_dt[:, ds].T.astype(f)),
            "cb": cbv,
            "b9": np.ascontiguousarray(b9.reshape(2, 128).T.astype(f)),
            "identb": np.eye(128, dtype=BF),
            "identf": np.eye(128, dtype=f),
        })
    return per_core, a_scales


TRACE = False
TRACE_TMPDIR = None
LAST_EXEC_NS = None
LAST_RES = None


def _get_prog(a_scales):
    key = ("prog_v21",)
    if key not in _prog_cache:
        nc = _build_program(a_scales)
        _split_excess_waits(nc)
        _prog_cache[key] = nc
    return _prog_cache[key]


def _run(nc, per_core):
    global LAST_EXEC_NS, LAST_RES
    res = run_bass_kernel_spmd(nc, per_core, core_ids=list(range(NCORES)),
                               trace=TRACE, tmpdir=TRACE_TMPDIR)
    LAST_EXEC_NS = res.exec_time_ns
    LAST_RES = res
    return res


def kernel(**inputs):
    per_core, a_scales = _host_prep(inputs)
    nc = _get_prog(a_scales)
    res = _run(nc, per_core)
    out = np.empty((B, L, VOCAB), np.float32)
    for c in range(NCORES):
        b, s = divmod(c, 4)
        out[b, LSH * s: LSH * (s + 1), :] = \
            res.results[c]["logits"][:, LSH * s: LSH * (s + 1)].T
    return out


# revision 39
# speedup vs baseline: 1.3783x; 1.3783x over previous
"""BysMamba Trainium2 kernel v2: 8-core SPMD bass/Tile implementation.

Sharding: core c = (batch b = c//4) x (d_inner shard s = c%4, 128 channels).
Replica groups [[0..3],[4..7]]. The fp32 residual h (256 x 2048 per batch,
replicated within each group) lives in SBUF for the whole kernel.

v2 structure (vs v1):
- in_proj+conv replicated: every core computes xi for ALL 512 channels (its
  conv weights are reordered so ch-tile 0 is the core's own shard), which
  makes x_proj fully local and removes the per-pass 48x2048 AllReduce.
- Each directional pass is split into two L-halves (1024 tokens) that are
  software-pipelined: while DVE runs the 16-state scan loop of half i, the
  PE/Act head of half i+2 runs, and the out-proj AllReduce + h-update of
  half i-1 rides DMA (cce-add with bf16->f32 cast) under the scans.
- Engine roles: DVE = scans (the only engine that can run them) + most
  dbx/gn mults + y2; Pool(gpsimd) = a few mults + cast/accum DMAs; Act =
  silu/exp/ln/dA; PE = all matmuls incl. the state-sum via identity lhsT
  and d_param*xi via diagonal lhsT.
- Per-layer weights streamed from DRAM, double-buffered, prefetched one
  layer ahead.
"""
import sys
import os

for _p in ("/opt/trn_rl_repo", "/root/.axon_site/_ro/trn_rl_repo"):
    if os.path.isdir(_p) and _p not in sys.path:
        sys.path.insert(0, _p)

import numpy as np
import ml_dtypes

import concourse.bass as bass
import concourse.tile as tile
from concourse import mybir
from concourse.bass_utils import run_bass_kernel_spmd

BF = ml_dtypes.bfloat16
F32 = mybir.dt.float32
BF16 = mybir.dt.bfloat16
I32 = mybir.dt.int32

B = 2
L = 2048
HC = 1024          # half-chunk length
DIM = 256
DIN = 512
NST = 16
VOCAB = 474
NM = 10
LPAD = 3
LT = L + LPAD
LSH = 512
NCORES = 8
GROUPS = [[0, 1, 2, 3], [4, 5, 6, 7]]
NPOOL = 5          # states whose dbx/gn mults run on Pool

_prog_cache = {}


def _split_excess_waits(nc, max_waits=1):
    """walrus here rejects >1 sync-wait per instruction; split the excess
    onto same-engine NoOps placed immediately before."""
    n = 0
    for fn in nc.m.functions:
        for blk in fn.blocks:
            out = []
            changed = False
            for inst in blk.instructions:
                si = inst.sync_info
                waits = list(si.on_wait) if si is not None and si.on_wait else []
                if len(waits) > max_waits:
                    extra = waits[:-max_waits]
                    si.on_wait = waits[-max_waits:]
                    for i in range(0, len(extra), max_waits):
                        out.append(mybir.InstNoOp(
                            name=f"{inst.name}-wsplit-{i}",
                            engine=inst.engine, ins=[], outs=[],
                            sync_info=mybir.SyncInfo(
                                on_wait=extra[i:i + max_waits], on_update=[]),
                        ))
                        n += 1
                    changed = True
                out.append(inst)
            if changed:
                blk.instructions = out
    return n


def _bcast_row_ap(dram_tile_ap, row, col0, width):
    """AP reading one DRAM row segment replicated across 128 partitions."""
    r = dram_tile_ap[row:row + 1, col0:col0 + width]
    return bass.AP(tensor=r.tensor, offset=r.offset, ap=[[0, 128], [1, width]])


def _build_program(a_scales):
    AOP = mybir.AluOpType
    AF = mybir.ActivationFunctionType

    nc = bass.Bass(num_devices=NCORES)

    def par(name, shape, dt):
        return nc.declare_dram_parameter(name, list(shape), dt, isOutput=False)

    t9 = par("t9", (9 * VOCAB, DIM), F32)
    idxp = par("idxp", (128, 36), I32)
    wconvD = par("wconvD", (128, NM * 4096), BF16)
    wzD = par("wzD", (128, NM * 256), BF16)
    wxD = par("wxD", (128, NM * 192), BF16)
    wdtD = par("wdtD", (16, NM * 128), BF16)
    woutD = par("woutD", (128, NM * 256), BF16)
    wdprmD = par("wdprmD", (128, NM * 128), BF16)
    lmh = par("lmh", (128, 2 * VOCAB), BF16)
    bdtp = par("bdt", (128, NM), F32)
    cbp = par("cb", (128, NM * 4), F32)
    b9p = par("b9", (128, 2), F32)
    identb = par("identb", (128, 128), BF16)
    identf = par("identf", (128, 128), F32)

    logits = nc.declare_dram_parameter("logits", [VOCAB, L], F32, isOutput=True)

    import contextlib
    with tile.TileContext(nc) as tc, contextlib.ExitStack() as ctx:
        persist = ctx.enter_context(tc.tile_pool(name="persist", bufs=1))
        wst = ctx.enter_context(tc.tile_pool(name="wst", bufs=2))
        ps = ctx.enter_context(tc.tile_pool(name="ps", bufs=2, space="PSUM"))
        bc = ctx.enter_context(tc.tile_pool(name="bc", bufs=4))
        wk = ctx.enter_context(tc.tile_pool(name="wk", bufs=2))
        fe = ctx.enter_context(tc.tile_pool(name="fe", bufs=3))
        dram = ctx.enter_context(tc.tile_pool(name="dram", bufs=3, space="DRAM"))

        def ld(param, shape, dt, tag):
            t = persist.tile(list(shape), dt, tag=tag, name=tag)
            nc.sync.dma_start(out=t[:], in_=param[:])
            return t

        lmh_s = ld(lmh, (128, 2 * VOCAB), BF16, "lmh_s")
        bdt_s = ld(bdtp, (128, NM), F32, "bdt_s")
        cb_s = ld(cbp, (128, NM * 4), F32, "cb_s")
        b9_s = ld(b9p, (128, 2), F32, "b9_s")
        idb_s = ld(identb, (128, 128), BF16, "idb_s")
        idf_s = ld(identf, (128, 128), F32, "idf_s")
        idx_s = ld(idxp, (128, 36), I32, "idx_s")

        h32 = [persist.tile([128, LT], F32, tag=f"h32_{k}", name=f"h32_{k}") for k in range(2)]
        hbf = [persist.tile([128, LT], BF16, tag=f"hbf_{k}", name=f"hbf_{k}") for k in range(2)]
        hrv = [persist.tile([128, LT], BF16, tag=f"hrv_{k}", name=f"hrv_{k}") for k in range(2)]
        for k in range(2):
            nc.vector.memset(h32[k][:], 0.0)
            nc.vector.memset(hbf[k][:], 0.0)
            nc.vector.memset(hrv[k][:], 0.0)

        xi4 = [persist.tile([128, L], BF16, tag=f"xi4_{m}", name=f"xi4_{m}")
               for m in range(4)]
        sz_t = persist.tile([128, L], BF16, tag="sz_t", name="sz_t")
        dl_t = persist.tile([128, L], BF16, tag="dl_t", name="dl_t")
        u_t = persist.tile([128, L], BF16, tag="u_t", name="u_t")
        carries = persist.tile([128, NST], F32, tag="carries", name="carries")
        outp = [persist.tile([128, L], BF16, tag=f"outp_{k}", name=f"outp_{k}") for k in range(2)]
        h0loc = [persist.tile([128, LSH], F32, tag=f"h0loc_{k}", name=f"h0loc_{k}") for k in range(2)]

        # ---- per-layer streamed weights ------------------------------------
        wcache = {}

        def load_weights(l):
            if l in wcache or l >= NM:
                return
            w = {}
            w["cv"] = wst.tile([128, 4096], BF16, tag="w_cv", name="w_cv")
            nc.sync.dma_start(out=w["cv"][:], in_=wconvD[:, l * 4096:(l + 1) * 4096])
            w["z"] = wst.tile([128, 256], BF16, tag="w_z", name="w_z")
            nc.sync.dma_start(out=w["z"][:], in_=wzD[:, l * 256:(l + 1) * 256])
            w["x"] = wst.tile([128, 192], BF16, tag="w_x", name="w_x")
            nc.sync.dma_start(out=w["x"][:], in_=wxD[:, l * 192:(l + 1) * 192])
            w["dt"] = wst.tile([16, 128], BF16, tag="w_dt", name="w_dt")
            nc.sync.dma_start(out=w["dt"][:], in_=wdtD[:, l * 128:(l + 1) * 128])
            w["o"] = wst.tile([128, 256], BF16, tag="w_o", name="w_o")
            nc.sync.dma_start(out=w["o"][:], in_=woutD[:, l * 256:(l + 1) * 256])
            w["dp"] = wst.tile([128, 128], BF16, tag="w_dp", name="w_dp")
            nc.sync.dma_start(out=w["dp"][:], in_=wdprmD[:, l * 128:(l + 1) * 128])
            wcache[l] = w

        load_weights(0)

        # ---- front-end -----------------------------------------------------
        ptt = ps.tile([128, HC], F32, tag="ps", name="fe_ps")
        for tau in range(4):
            acc = fe.tile([128, DIM], F32, tag="feacc", name="feacc")
            for j in range(9):
                g = fe.tile([128, DIM], F32, tag="feg", name="feg")
                nc.gpsimd.indirect_dma_start(
                    out=g[:], out_offset=None, in_=t9[:],
                    in_offset=bass.IndirectOffsetOnAxis(
                        ap=idx_s[:, tau * 9 + j: tau * 9 + j + 1], axis=0),
                )
                if j == 0:
                    nc.vector.tensor_copy(out=acc[:], in_=g[:])
                else:
                    nc.vector.tensor_tensor(out=acc[:], in0=acc[:], in1=g[:],
                                             op=AOP.add)
            for dh in range(2):
                blk = tau * 2 + dh
                nc.tensor.transpose(
                    out=ptt[:, blk * 128:(blk + 1) * 128],
                    in_=acc[:, dh * 128:(dh + 1) * 128],
                    identity=idf_s[:])
                nc.vector.tensor_scalar(
                    out=h0loc[dh][:, tau * 128:(tau + 1) * 128],
                    in0=ptt[:, blk * 128:(blk + 1) * 128],
                    scalar1=b9_s[:, dh:dh + 1], scalar2=None, op0=AOP.add)

        agi = dram.tile([2, 128, LSH], F32, tag="agi", name="agi", bufs=1)
        ago = dram.tile([4, 2, 128, LSH], F32, tag="ago", name="ago", bufs=1)
        for k in range(2):
            nc.sync.dma_start(out=agi[k], in_=h0loc[k][:])
        nc.gpsimd.collective_compute(
            "AllGather", AOP.bypass, replica_groups=GROUPS,
            ins=[agi.opt()], outs=[ago.opt()])
        for g in range(4):
            for k in range(2):
                nc.sync.dma_start(
                    out=h32[k][:, LPAD + g * LSH: LPAD + (g + 1) * LSH],
                    in_=ago[g, k])
        for k in range(2):
            nc.vector.tensor_copy(out=hbf[k][:], in_=h32[k][:])

        # ---- half-pass building blocks ------------------------------------

        def head(l, hb, c, first_of_layer):
            """PE/Act/DMA head for half c: in_proj(all ch) -> silu -> x_proj
            -> dbc -> dt -> e/ln. No DVE ops."""
            w = wcache[l]
            if first_of_layer:
                load_weights(l + 1)
            c0 = c * HC
            for m in range(4):
                pxc = ps.tile([128, HC], F32, tag="ps", name="pxc")
                for kt in range(2):
                    for j in range(4):
                        lt = w["cv"][:, m * 1024 + (j * 2 + kt) * 128:
                                     m * 1024 + (j * 2 + kt) * 128 + 128]
                        for nt in range(2):
                            nc.tensor.matmul(
                                out=pxc[:, nt * 512:(nt + 1) * 512],
                                lhsT=lt,
                                rhs=hb[kt][:, c0 + nt * 512 + j:
                                           c0 + nt * 512 + j + 512],
                                start=(kt == 0 and j == 0),
                                stop=(kt == 1 and j == 3))
                nc.scalar.activation(out=xi4[m][:, c0:c0 + HC], in_=pxc[:],
                                     func=AF.Silu,
                                     bias=cb_s[:, l * 4 + m:l * 4 + m + 1],
                                     scale=1.0)
            pz = ps.tile([128, HC], F32, tag="ps", name="pz")
            for nt in range(2):
                for kt in range(2):
                    nc.tensor.matmul(
                        out=pz[:, nt * 512:(nt + 1) * 512],
                        lhsT=w["z"][:, kt * 128:(kt + 1) * 128],
                        rhs=hb[kt][:, LPAD + c0 + nt * 512:
                                   LPAD + c0 + nt * 512 + 512],
                        start=(kt == 0), stop=(kt == 1))
            nc.scalar.activation(out=sz_t[:, c0:c0 + HC], in_=pz[:],
                                 func=AF.Silu, scale=1.0)
            pxp = ps.tile([128, HC], F32, tag="ps", name="pxp")
            for nt in range(2):
                for m in range(4):
                    nc.tensor.matmul(
                        out=pxp[:48, nt * 512:(nt + 1) * 512],
                        lhsT=w["x"][:, m * 48:(m + 1) * 48],
                        rhs=xi4[m][:, c0 + nt * 512: c0 + nt * 512 + 512],
                        start=(m == 0), stop=(m == 3))
            # dbc: PSUM -> SBUF bf16 (Act copy) -> DRAM for row broadcasts
            dbc_sb = wk.tile([48, HC], BF16, tag="dbc_sb", name="dbc_sb")
            nc.scalar.copy(out=dbc_sb[:], in_=pxp[:48, :])
            dbcD = dram.tile([48, HC], BF16, tag="dbcD", name="dbcD")
            nc.sync.dma_start(out=dbcD[:], in_=dbc_sb[:])
            dt_sb = dbc_sb[:16, :]
            pdt = ps.tile([128, HC], F32, tag="ps", name="pdt")
            for nt in range(2):
                nc.tensor.matmul(out=pdt[:, nt * 512:(nt + 1) * 512],
                                 lhsT=w["dt"][:],
                                 rhs=dt_sb[:, nt * 512:(nt + 1) * 512],
                                 start=True, stop=True)
            e_b = wk.tile([128, HC], BF16, tag="e_b", name="e_b")
            nc.scalar.activation(out=e_b[:], in_=pdt[:], func=AF.Exp,
                                 bias=bdt_s[:, l:l + 1], scale=1.0)
            nc.scalar.activation(out=dl_t[:, c0:c0 + HC], in_=e_b[:],
                                 func=AF.Ln, bias=1.0, scale=1.0)
            return dbcD

        def da_block(l, c):
            """16 Act exps for half c (deep-buffered da tiles)."""
            c0 = c * HC
            das = []
            for n in range(NST):
                da = wk.tile([128, HC], BF16, tag="da", name="da", bufs=10)
                nc.scalar.activation(out=da[:], in_=dl_t[:, c0:c0 + HC],
                                     func=AF.Exp, scale=float(a_scales[l][n]))
                das.append(da)
            return das

        def states_tail(l, c, dbcD, das, oinD):
            """DVE/Pool state loop for half c + y2/out-proj."""
            w = wcache[l]
            c0 = c * HC
            nc.vector.tensor_tensor(out=u_t[:, c0:c0 + HC],
                                    in0=dl_t[:, c0:c0 + HC],
                                    in1=xi4[0][:, c0:c0 + HC], op=AOP.mult)
            py = ps.tile([128, HC], F32, tag="py", name="py")
            for n in range(NST):
                pool_state = (n >= NST - NPOOL)
                eng = nc.gpsimd if pool_state else nc.vector
                sfx = "p" if pool_state else "v"
                bbc = bc.tile([128, HC], BF16, tag="bbc", name="bbc")
                nc.sync.dma_start(out=bbc[:],
                                  in_=_bcast_row_ap(dbcD, 16 + n, 0, HC))
                cbc = bc.tile([128, HC], BF16, tag="cbc", name="cbc")
                nc.sync.dma_start(out=cbc[:],
                                  in_=_bcast_row_ap(dbcD, 32 + n, 0, HC))
                dbx = wk.tile([128, HC], BF16, tag=f"dbx_{sfx}", name="dbx")
                eng.tensor_tensor(out=dbx[:], in0=u_t[:, c0:c0 + HC],
                                  in1=bbc[:], op=AOP.mult)
                hn = wk.tile([128, HC], BF16, tag="hn", name="hn")
                nc.vector.tensor_tensor_scan(
                    out=hn[:], data0=das[n][:], data1=dbx[:],
                    initial=(0.0 if c == 0 else carries[:, n:n + 1]),
                    op0=AOP.mult, op1=AOP.add)
                if c == 0:
                    nc.vector.tensor_copy(out=carries[:, n:n + 1],
                                          in_=hn[:, HC - 1:HC])
                gn = wk.tile([128, HC], BF16, tag=f"gn_{sfx}", name="gn")
                eng.tensor_tensor(out=gn[:], in0=hn[:], in1=cbc[:],
                                  op=AOP.mult)
                for nt in range(2):
                    nc.tensor.matmul(
                        out=py[:, nt * 512:(nt + 1) * 512],
                        lhsT=idb_s[:],
                        rhs=gn[:, nt * 512:(nt + 1) * 512],
                        start=(n == 0), stop=False)
            for nt in range(2):
                nc.tensor.matmul(
                    out=py[:, nt * 512:(nt + 1) * 512],
                    lhsT=w["dp"][:],
                    rhs=xi4[0][:, c0 + nt * 512: c0 + nt * 512 + 512],
                    start=False, stop=(nt == 1))
            y2 = wk.tile([128, HC], BF16, tag="y2", name="y2")
            nc.vector.tensor_tensor(out=y2[:], in0=py[:],
                                    in1=sz_t[:, c0:c0 + HC], op=AOP.mult)
            for mt in range(2):
                po = ps.tile([128, HC], F32, tag="ps", name="po")
                for nt in range(2):
                    nc.tensor.matmul(out=po[:, nt * 512:(nt + 1) * 512],
                                     lhsT=w["o"][:, mt * 128:(mt + 1) * 128],
                                     rhs=y2[:, nt * 512:(nt + 1) * 512],
                                     start=True, stop=True)
                pob = wk.tile([128, HC], BF16, tag="pob", name="pob")
                nc.scalar.copy(out=pob[:], in_=po[:])
                nc.sync.dma_start(out=oinD[mt], in_=pob[:])

        def arh(l, c, oinD, mode):
            """AllReduce of the out halves + h update."""
            c0 = c * HC
            ooutD = dram.tile([2, 128, HC], BF16, tag="ooutD", name="ooutD")
            nc.gpsimd.collective_compute(
                "AllReduce", AOP.add, replica_groups=GROUPS,
                ins=[oinD.opt()], outs=[ooutD.opt()])
            if mode == "fwd":
                for k in range(2):
                    nc.gpsimd.dma_start(
                        out=h32[k][:, LPAD + c0: LPAD + c0 + HC],
                        in_=ooutD[k], accum_op=AOP.add)
                    nc.gpsimd.dma_start(
                        out=hbf[k][:, LPAD + c0: LPAD + c0 + HC],
                        in_=h32[k][:, LPAD + c0: LPAD + c0 + HC])
            else:  # bwd: stage; reversed add at layer end
                for k in range(2):
                    nc.sync.dma_start(out=outp[k][:, c0:c0 + HC],
                                      in_=ooutD[k])

        def bwd_finish():
            for k in range(2):
                nc.vector.tensor_tensor(
                    out=h32[k][:, LPAD:], in0=h32[k][:, LPAD:],
                    in1=outp[k][:, L - 1::-1], op=AOP.add)
                nc.gpsimd.dma_start(out=hbf[k][:, LPAD:],
                                    in_=h32[k][:, LPAD:])

        def refresh_hrv():
            for k in range(2):
                nc.vector.tensor_copy(out=hrv[k][:, LPAD:],
                                      in_=hbf[k][:, LT - 1: LPAD - 1: -1])

        # ---- schedule: 24 half-passes, software-pipelined ------------------
        sched = []
        for l in range(NM):
            bidir = (l == 0 or l == NM - 1)
            if bidir:
                sched += [(l, "fwd", 0), (l, "fwd", 1),
                          (l, "bwd", 0), (l, "bwd", 1)]
            else:
                sched += [(l, "fwd", 0), (l, "fwd", 1)]
        nhp = len(sched)
        last_of_layer = {}
        for i, (l, mode, c) in enumerate(sched):
            last_of_layer[l] = i
        bidir_layers = {0, NM - 1}

        state = {}

        def issue_head(i):
            l, mode, c = sched[i]
            first = (c == 0 and mode == "fwd")
            dbcD = head(l, hbf if mode == "fwd" else hrv, c, first)
            das = da_block(l, c)
            oinD = dram.tile([2, 128, HC], BF16, tag="oinD", name="oinD")
            state[i] = (dbcD, das, oinD)

        def head_allowed(j, i):
            """head(j) may be issued at end of iteration i?"""
            lj = sched[j][0]
            prev = lj - 1
            if prev >= 0 and prev in bidir_layers and i < last_of_layer[prev]:
                return False
            return True

        next_head = 0

        def pump_heads(i):
            nonlocal next_head
            while next_head < nhp and next_head <= i + 2 \
                    and head_allowed(next_head, i):
                issue_head(next_head)
                next_head += 1

        # l=0 is bidirectional: snapshot the FE h for its bwd passes before
        # any fwd h-update is issued.
        refresh_hrv()
        pump_heads(-1)  # issues heads 0 and 1
        for i in range(nhp):
            l, mode, c = sched[i]
            if l in bidir_layers and mode == "fwd" and c == 0 and l > 0:
                # snapshot the layer input for the bwd passes: the previous
                # layer's h is fully updated (its last arh was issued in the
                # previous iteration) and none of this layer's fwd updates
                # are issued yet.
                refresh_hrv()
            dbcD, das, oinD = state.pop(i)
            states_tail(l, c, dbcD, das, oinD)
            arh(l, c, oinD, mode)
            if mode == "bwd" and c == 1:
                bwd_finish()
            pump_heads(i)

        # ---- lm_head over full L (host slices per core) --------------------
        for mt in range(4):
            m0 = mt * 128
            msz = min(128, VOCAB - m0)
            for nt in range(4):
                plh = ps.tile([128, HC], F32, tag="ps", name="plh")
                for kt in range(2):
                    nc.tensor.matmul(
                        out=plh[:msz, :512],
                        lhsT=lmh_s[:, kt * VOCAB + m0: kt * VOCAB + m0 + msz],
                        rhs=hbf[kt][:, LPAD + nt * 512: LPAD + nt * 512 + 512],
                        start=(kt == 0), stop=(kt == 1))
                lout = wk.tile([128, 512], F32, tag="lout", name="lout")
                nc.vector.tensor_copy(out=lout[:msz, :], in_=plh[:msz, :512])
                nc.sync.dma_start(
                    out=logits[m0:m0 + msz, nt * 512:(nt + 1) * 512],
                    in_=lout[:msz, :])

    return nc


# --------------------------------------------------------------------------
def _host_prep(inputs):
    f = np.float32
    x = np.asarray(inputs["x"]).astype(np.int64).reshape(B, L, 9)
    emb = np.asarray(inputs["emb"], f)
    c2w = np.asarray(inputs["conv2d_w"], f)
    c2b = np.asarray(inputs["conv2d_b"], f)
    w_in = np.asarray(inputs["w_in"], f)
    conv_w = np.asarray(inputs["conv_w"], f)
    conv_b = np.asarray(inputs["conv_b"], f)
    w_x = np.asarray(inputs["w_x"], f)
    w_dt = np.asarray(inputs["w_dt"], f)
    b_dt = np.asarray(inputs["b_dt"], f)
    a_log = np.asarray(inputs["a_log"], f)
    d_param = np.asarray(inputs["d_param"], f)
    w_out = np.asarray(inputs["w_out"], f)
    lm_head = np.asarray(inputs["lm_head"], f)

    t9 = np.empty((9, VOCAB, DIM), f)
    for j in range(9):
        i, jj = divmod(j, 3)
        t9[j] = 0.5 * (emb @ c2w[:, :, i, jj].T)
    t9[4] += 0.5 * emb
    t9f = np.ascontiguousarray(t9.reshape(9 * VOCAB, DIM))
    b9 = 0.5 * c2b

    a_scales = [[float(-np.exp(a_log[l, 0, n])) for n in range(NST)]
                for l in range(NM)]

    per_core = []
    for cid in range(NCORES):
        b, s = divmod(cid, 4)
        ds = slice(128 * s, 128 * s + 128)
        dglob = np.arange(128 * s, 128 * s + 128)

        tok = np.arange(LSH * s, LSH * (s + 1))
        idx = (np.arange(9)[None, :] * VOCAB + x[b][tok]).astype(np.int32)
        idxp = np.zeros((128, 36), np.int32)
        for tau in range(4):
            idxp[:, tau * 9:(tau + 1) * 9] = idx[tau * 128:(tau + 1) * 128]

        wconv = np.zeros((128, NM * 4096), BF)
        wzv = np.zeros((128, NM * 256), BF)
        wxv = np.zeros((128, NM * 192), BF)
        wdtv = np.zeros((16, NM * 128), BF)
        woutv = np.zeros((128, NM * 256), BF)
        wdprmv = np.zeros((128, NM * 128), BF)
        cbv = np.zeros((128, NM * 4), f)
        for l in range(NM):
            wi_all = w_in[l][:DIN]            # (512, 256)
            wzr = w_in[l][DIN:][ds]           # own z rows (128, 256)
            cw_all = conv_w[l]                # (512, 4)
            for m in range(4):
                g = (s + m) % 4
                gs = slice(128 * g, 128 * g + 128)
                wi = wi_all[gs]               # (128, 256)
                cw = cw_all[gs]               # (128, 4)
                for j in range(4):
                    for kt in range(2):
                        blkc = l * 4096 + m * 1024 + (j * 2 + kt) * 128
                        wconv[:, blkc:blkc + 128] = (
                            cw[:, j][None, :]
                            * wi[:, kt * 128:kt * 128 + 128].T)
                wxv[:, l * 192 + m * 48: l * 192 + (m + 1) * 48] = \
                    w_x[l][:, gs].T
                cbv[:, l * 4 + m] = conv_b[l][gs]
            for kt in range(2):
                wzv[:, l * 256 + kt * 128: l * 256 + (kt + 1) * 128] = \
                    wzr[:, kt * 128:kt * 128 + 128].T
            wdtv[:, l * 128:(l + 1) * 128] = w_dt[l][dglob].T
            sc = 0.5 if (l == 0 or l == NM - 1) else 1.0
            woutv[:, l * 256:(l + 1) * 256] = sc * w_out[l][:, dglob].T
            wdprmv[:, l * 128:(l + 1) * 128] = np.diag(d_param[l][dglob])

        lmhv = np.zeros((128, 2 * VOCAB), BF)
        for kt in range(2):
            lmhv[:, kt * VOCAB:(kt + 1) * VOCAB] = \
                lm_head[:, kt * 128:(kt + 1) * 128].T

        per_core.append({
            "t9": t9f,
            "idxp": idxp,
            "wconvD": wconv, "wzD": wzv, "wxD": wxv, "wdtD": wdtv,
            "woutD": woutv, "wdprmD": wdprmv,
            "lmh": lmhv,
            "bdt": np.ascontiguousarray(b_dt[:, ds].T.astype(f)),
            "cb": cbv,
            "b9": np.ascontiguousarray(b9.reshape(2, 128).T.astype(f)),
            "identb": np.eye(128, dtype=BF),
            "identf": np.eye(128, dtype=f),
        })
    return per_core, a_scales


TRACE = False
TRACE_TMPDIR = None
LAST_EXEC_NS = None
LAST_RES = None


def _get_prog(a_scales):
    key = ("prog_v22",)
    if key not in _prog_cache:
        nc = _build_program(a_scales)
        _split_excess_waits(nc)
        _prog_cache[key] = nc
    return _prog_cache[key]


def _run(nc, per_core):
    global LAST_EXEC_NS, LAST_RES
    res = run_bass_kernel_spmd(nc, per_core, core_ids=list(range(NCORES)),
                               trace=TRACE, tmpdir=TRACE_TMPDIR)
    LAST_EXEC_NS = res.exec_time_ns
    LAST_RES = res
    return res


def kernel(**inputs):
    per_core, a_scales = _host_prep(inputs)
    nc = _get_prog(a_scales)
    res = _run(nc, per_core)
    out = np.empty((B, L, VOCAB), np.float32)
    for c in range(NCORES):
        b, s = divmod(c, 4)
        out[b, LSH * s: LSH * (s + 1), :] = \
            res.results[c]["logits"][:, LSH * s: LSH * (s + 1)].T
    return out
